# revision 20
# baseline (speedup 1.0000x reference)
"""KBertGATEnricher Trainium2 kernel.

Sharding: data-parallel over batch (8 batches -> 8 cores) for embedding+GAT,
then AllGather of the f16 head features, then vocab-column-parallel output
Linear (f16 matmuls, kt-major so the PE streams back-to-back into all 8 PSUM
banks) + global log_softmax. The per-token sum-exp AllReduce is split into
groups so the normalize (in-place on vp) + f16 store tail overlaps compute;
the f16 output is upconverted to f32 on the host.

Self-contained: hardcodes all shapes; only imports the system-installed
concourse runtime.
"""

import os
import sys

sys.path.insert(0, "/opt/trn_rl_repo")

import numpy as np

from concourse import bass, bacc, mybir, tile
from concourse.bass_utils import run_bass_kernel_spmd

F32 = mybir.dt.float32
F16 = mybir.dt.float16
F8 = mybir.dt.float8e4
U8 = mybir.dt.uint8
PM_DR = mybir.MatmulPerfMode.DoubleRow

B, N, D, H, F, V = 8, 256, 768, 4, 128, 30522
NCORES = 8
VS = 3816          # per-core vocab columns (8*3816 = 30528, 6 pad cols)
VPAD = VS * NCORES
LN_EPS = 1e-12
ALPHA = 0.01       # leaky relu slope
MASK_NEG = -50.0   # masked attention logit (exact, LUT-safe)
NKT = D // 128     # 6 hidden k-tiles for the GAT matmuls
NM = (B * N) // 128  # 16 token m-tiles
CHUNKS = [(c0, min(512, VS - c0)) for c0 in range(0, VS, 512)]  # 8 chunks
WT = [min(1024, VS - t * 1024) for t in range(4)]  # zp tile widths
# AllReduce groups: big early (pipeline warmup), tiny last (short tail)
GROUP_MS = [[0, 1], [2, 3], [4, 5], [6, 7], [8, 9], [10, 11], [12, 13, 14, 15]]
NG = len(GROUP_MS)

AX = mybir.AxisListType
AF = mybir.ActivationFunctionType
OP = mybir.AluOpType

_NC_CACHE = {}


def _build(with_ln_b: bool, with_out_b: bool):
    """Build the SPMD Bass program (identical on all 8 cores)."""
    nc = bacc.Bacc(
        "TRN2",
        target_bir_lowering=False,
        debug=False,
        enable_asserts=False,
        num_devices=NCORES,
    )

    # ---- per-core I/O --------------------------------------------------
    xpre = nc.dram_tensor("xpre", [N, D], F32, kind="ExternalInput").ap()
    maskt = nc.dram_tensor("maskt", [N, N], U8, kind="ExternalInput").ap()
    waug = nc.dram_tensor("waug", [D, H * 130], F16, kind="ExternalInput").ap()
    wst = nc.dram_tensor("wst", [4, 128, VS], F16, kind="ExternalInput").ap()
    pad = nc.dram_tensor("pad", [128, 1], F32, kind="ExternalInput").ap()
    if with_ln_b:
        brow = nc.dram_tensor("brow", [1, H * 130], F16, kind="ExternalInput").ap()
    if with_out_b:
        bvoc = nc.dram_tensor("bvoc", [1, VS], F16, kind="ExternalInput").ap()
    out = nc.dram_tensor("out", [B * N, VS], F16, kind="ExternalOutput").ap()

    rg = [list(range(NCORES))]

    with tile.TileContext(nc) as tc:
        # ---- persistent SBUF ------------------------------------------
        with (
            tc.tile_pool(name="wpool", bufs=1) as wpool,
            tc.tile_pool(name="catf_pool", bufs=1) as catf_pool,
            tc.tile_pool(name="dram", bufs=1, space="DRAM") as dram,
        ):
            w_sb = [
                wpool.tile([128, VS], F16, tag=f"w{k}", name=f"w{k}")
                for k in range(4)
            ]
            for k in range(4):
                nc.sync.dma_start(out=w_sb[k][:], in_=wst[k, :, :])
            catf = [
                catf_pool.tile([128, B * N], F16, tag=f"catf{k}", name=f"catf{k}")
                for k in range(4)
            ]
            if with_out_b:
                bvoc_sb = wpool.tile([1, VS], F16, tag="bvoc")
                nc.sync.dma_start(out=bvoc_sb[:], in_=bvoc[:, :])
                ones1v = wpool.tile([1, 128], F16, tag="ones1v")
                nc.vector.memset(ones1v[:], 1.0)

            cc_in = dram.tile([H * F, N], F16, tag="cci", name="cci")
            cc_out = dram.tile([NCORES, H * F, N], F16, tag="cco", name="cco",
                               addr_space="Shared")
            sum_in = [
                dram.tile([128, len(GROUP_MS[g])], F32, tag=f"sin{g}", name=f"sin{g}")
                for g in range(NG)
            ]
            sum_out = [
                dram.tile([128, len(GROUP_MS[g])], F32, tag=f"sout{g}", name=f"sout{g}",
                          addr_space="Shared")
                for g in range(NG)
            ]

            # ==== phase A: embedding LN + GAT (own batch) ==============
            with (
                tc.tile_pool(name="pa", bufs=1) as pa,
                tc.tile_pool(name="pa_tmp", bufs=3) as pa_tmp,
                tc.tile_pool(name="ps_a", bufs=2, space="PSUM") as ps_a,
            ):
                idw = pa.tile([128, 128], F16, tag="idw")
                bass_masks_identity(nc, idw[:])
                ones1 = pa.tile([1, 128], F32, tag="ones1")
                nc.vector.memset(ones1[:], 1.0)
                if with_ln_b:
                    ones1h = pa.tile([1, 128], F16, tag="ones1h")
                    nc.vector.memset(ones1h[:], 1.0)
                    ones_n = pa.tile([1, N], F16, tag="ones_n")
                    nc.vector.memset(ones_n[:], 1.0)
                negt = pa.tile([128, N], F32, tag="negt")
                nc.vector.memset(negt[:], MASK_NEG)
                eps_sb = pa.tile([128, 1], F32, tag="eps_sb")
                nc.vector.memset(eps_sb[:], LN_EPS)

                waug_sb = [
                    pa.tile([128, H * 130], F16, tag=f"waug{kt}", name=f"waug{kt}") for kt in range(NKT)
                ]
                for kt in range(NKT):
                    nc.sync.dma_start(
                        out=waug_sb[kt][:], in_=waug[kt * 128 : (kt + 1) * 128, :]
                    )
                mask_sb = [pa.tile([128, N], U8, tag=f"mask{j}", name=f"mask{j}") for j in range(2)]
                for jt in range(2):
                    nc.sync.dma_start(
                        out=mask_sb[jt][:], in_=maskt[jt * 128 : (jt + 1) * 128, :]
                    )
                if with_ln_b:
                    brow_sb = pa.tile([1, H * 130], F16, tag="brow")
                    nc.sync.dma_start(out=brow_sb[:], in_=brow[:, :])

                # ---- LayerNorm (tokens on partitions) -----------------
                xn_sb = [pa.tile([128, D], F16, tag=f"xn{m}", name=f"xn{m}") for m in range(2)]
                for m in range(2):
                    xp = pa_tmp.tile([128, D], F32, tag="xp")
                    nc.sync.dma_start(
                        out=xp[:], in_=xpre[m * 128 : (m + 1) * 128, :]
                    )
                    nmu = pa_tmp.tile([128, 1], F32, tag="nmu")
                    nc.vector.tensor_reduce(
                        out=nmu[:], in_=xp[:], axis=AX.X, op=OP.add, negate=True
                    )
                    nc.vector.tensor_scalar_mul(nmu[:], nmu[:], 1.0 / D)
                    xc = pa_tmp.tile([128, D], F32, tag="xc")
                    nc.vector.tensor_scalar_add(xc[:], xp[:], nmu[:, 0:1])
                    sq = pa_tmp.tile([128, D], F32, tag="sq")
                    ssum = pa_tmp.tile([128, 1], F32, tag="ssum")
                    nc.scalar.activation(
                        sq[:], xc[:], AF.Square, accum_out=ssum[:, 0:1]
                    )
                    sd = pa_tmp.tile([128, 1], F32, tag="sd")
                    nc.scalar.activation(
                        sd[:], ssum[:], AF.Sqrt, bias=eps_sb[:, 0:1], scale=1.0 / D
                    )
                    rstd = pa_tmp.tile([128, 1], F32, tag="rstd")
                    nc.vector.reciprocal(rstd[:], sd[:])
                    nc.vector.tensor_scalar_mul(xn_sb[m][:], xc[:], rstd[:, 0:1])

                # ---- transpose xn -> xT[kt] [128 hid, 256 tok] --------
                xt_sb = [pa.tile([128, N], F16, tag=f"xt{kt}", name=f"xt{kt}") for kt in range(NKT)]
                for kt in range(NKT):
                    for m in range(2):
                        ptr = ps_a.tile([128, 128], F16, tag="ptr")
                        nc.tensor.transpose(
                            ptr[:], xn_sb[m][:, kt * 128 : (kt + 1) * 128], idw[:]
                        )
                        nc.scalar.copy(
                            xt_sb[kt][:, m * 128 : (m + 1) * 128], ptr[:]
                        )

                # ---- per-head GAT -------------------------------------
                wh_sb = [
                    [pa.tile([128, 128], F16, tag=f"wh{h}_{m}", name=f"wh{h}_{m}") for m in range(2)]
                    for h in range(H)
                ]
                s2c = [
                    [pa.tile([128, 1], F32, tag=f"s2{h}_{m}", name=f"s2{h}_{m}") for m in range(2)]
                    for h in range(H)
                ]
                s1r = [pa.tile([1, N], F32, tag=f"s1r{h}", name=f"s1r{h}") for h in range(H)]
                att = [
                    [pa.tile([128, N], F16, tag=f"att{h}_{m}", name=f"att{h}_{m}") for m in range(2)]
                    for h in range(H)
                ]
                cat_sb = [pa.tile([128, N], F16, tag=f"cat{h}", name=f"cat{h}") for h in range(H)]

                for h in range(H):
                    c0 = h * 130
                    # Wh (+ s1,s2 fused columns)
                    for m in range(2):
                        pwh = ps_a.tile([128, 130], F32, tag="pwh")
                        for kt in range(NKT):
                            nc.tensor.matmul(
                                pwh[:],
                                xt_sb[kt][:, m * 128 : (m + 1) * 128],
                                waug_sb[kt][:, c0 : c0 + 130],
                                start=(kt == 0),
                                stop=(kt == NKT - 1) and not with_ln_b,
                            )
                        if with_ln_b:
                            nc.tensor.matmul(
                                pwh[:],
                                ones1h[:],
                                brow_sb[:, c0 : c0 + 130],
                                start=False,
                                stop=True,
                            )
                        nc.scalar.copy(wh_sb[h][m][:], pwh[:, 0:128])
                        nc.scalar.copy(s2c[h][m][:], pwh[:, 129:130])

                    # s1 row: c1^T @ xT  -> [1, 256]
                    ps1 = ps_a.tile([1, N], F32, tag="ps1", bufs=1)
                    for kt in range(NKT):
                        nc.tensor.matmul(
                            ps1[:],
                            waug_sb[kt][:, c0 + 128 : c0 + 129],
                            xt_sb[kt][:],
                            start=(kt == 0),
                            stop=(kt == NKT - 1) and not with_ln_b,
                        )
                    if with_ln_b:
                        nc.tensor.matmul(
                            ps1[:],
                            brow_sb[:, c0 + 128 : c0 + 129],
                            ones_n[:],
                            start=False,
                            stop=True,
                        )
                    nc.scalar.copy(s1r[h][:], ps1[:])

                    # attention scores + column softmax (over i = free dim)
                    for jt in range(2):
                        pet = ps_a.tile([128, N], F32, tag="pet")
                        nc.tensor.matmul(
                            pet[:], ones1[:], s1r[h][:], start=True, stop=True
                        )
                        et = pa_tmp.tile([128, N], F32, tag="et")
                        nc.vector.tensor_scalar_add(et[:], pet[:], s2c[h][jt][:, 0:1])
                        lr = pa_tmp.tile([128, N], F32, tag="lr")
                        nc.vector.scalar_tensor_tensor(
                            lr[:], et[:], ALPHA, et[:], OP.mult, OP.max
                        )
                        nc.vector.copy_predicated(lr[:], mask_sb[jt][:], negt[:])
                        nmax = pa_tmp.tile([128, 1], F32, tag="nmax")
                        nc.vector.tensor_reduce(
                            out=nmax[:], in_=lr[:], axis=AX.X, op=OP.max, negate=True
                        )
                        ex = pa_tmp.tile([128, N], F16, tag="ex")
                        asum = pa_tmp.tile([128, 1], F32, tag="asum")
                        nc.scalar.activation(
                            ex[:],
                            lr[:],
                            AF.Exp,
                            bias=nmax[:, 0:1],
                            accum_out=asum[:, 0:1],
                        )
                        rec = pa_tmp.tile([128, 1], F32, tag="rec")
                        nc.vector.reciprocal(rec[:], asum[:])
                        nc.vector.tensor_scalar_mul(
                            att[h][jt][:], ex[:], rec[:, 0:1]
                        )

                    # hp^T = Wh^T @ att^T, then elu -> catT rows of head h
                    php = ps_a.tile([128, N], F32, tag="php", bufs=1)
                    for jt in range(2):
                        nc.tensor.matmul(
                            php[:],
                            wh_sb[h][jt][:],
                            att[h][jt][:],
                            start=(jt == 0),
                            stop=(jt == 1),
                        )
                    hneg = pa_tmp.tile([128, N], F32, tag="hneg")
                    nc.vector.tensor_scalar_min(hneg[:], php[:], 0.0)
                    he = pa_tmp.tile([128, N], F32, tag="he")
                    nc.scalar.activation(he[:], hneg[:], AF.Exp)
                    r1 = pa_tmp.tile([128, N], F32, tag="r1")
                    nc.vector.tensor_scalar(r1[:], he[:], -1.0, 1.0, OP.mult, OP.add)
                    nc.vector.scalar_tensor_tensor(
                        cat_sb[h][:], php[:], 0.0, r1[:], OP.max, OP.subtract
                    )
                    nc.sync.dma_start(
                        out=cc_in[h * 128 : (h + 1) * 128, :], in_=cat_sb[h][:]
                    )

            # ==== AllGather cat, assemble GEMM lhsT tiles ==============
            nc.gpsimd.collective_compute(
                "AllGather",
                OP.bypass,
                replica_groups=rg,
                ins=[cc_in.opt()],
                outs=[cc_out.opt()],
            )
            for kt in range(4):
                for r in range(NCORES):
                    nc.sync.dma_start(
                        out=catf[kt][:, r * N : (r + 1) * N],
                        in_=cc_out[r, kt * 128 : (kt + 1) * 128, :],
                    )

            # ==== vocab-parallel output linear + softmax stats =========
            with (
                tc.tile_pool(name="vp_pool", bufs=14) as vp_pool,
                tc.tile_pool(name="wide_tmp", bufs=2) as wide_tmp,
                tc.tile_pool(name="dum_pool", bufs=2) as dum_pool,
                tc.tile_pool(name="stat", bufs=1) as stat,
                tc.tile_pool(name="ps_z", bufs=4, space="PSUM") as ps_z,
            ):
                vp = {}
                sums = stat.tile([128, NM], F32, tag="sums")
                negone = stat.tile([128, 1], F32, tag="negone")
                nc.vector.memset(negone[:], -1.0)
                pad_sb = stat.tile([128, 1], F32, tag="pad_sb")
                nc.sync.dma_start(out=pad_sb[:], in_=pad[:, :])
                nlogl1 = stat.tile([128, NM], F32, tag="nlogl1")

                def emit_mtile(m):
                    vp[m] = vp_pool.tile([128, VS], F16, tag="vp", name=f"vp{m}")
                    zps = [ps_z.tile([128, 1024], F32, tag="zp", name=f"zp{m}_{t}") for t in range(4)]
                    for kt in range(4):
                        for ci, (c0, cw) in enumerate(CHUNKS):
                            t, h = divmod(ci, 2)
                            nc.tensor.matmul(
                                zps[t][:, h * 512 : h * 512 + cw],
                                catf[kt][:, m * 128 : (m + 1) * 128],
                                w_sb[kt][:, c0 : c0 + cw],
                                start=(kt == 0),
                                stop=(kt == 3) and not with_out_b,
                            )
                    if with_out_b:
                        for ci, (c0, cw) in enumerate(CHUNKS):
                            t, h = divmod(ci, 2)
                            nc.tensor.matmul(
                                zps[t][:, h * 512 : h * 512 + cw],
                                ones1v[:],
                                bvoc_sb[:, c0 : c0 + cw],
                                start=False,
                                stop=True,
                            )
                    for t in range(4):
                        w = WT[t]
                        e0 = wide_tmp.tile([128, 1024], F16, tag="e0")
                        nc.scalar.activation(e0[:, 0:w], zps[t][:, 0:w], AF.Exp)
                        tmin = wide_tmp.tile([128, 1024], F16, tag="tmin")
                        nc.vector.tensor_scalar_min(tmin[:, 0:w], e0[:, 0:w], 1.0)
                        nc.vector.scalar_tensor_tensor(
                            vp[m][:, t * 1024 : t * 1024 + w],
                            zps[t][:, 0:w],
                            0.0,
                            tmin[:, 0:w],
                            OP.max,
                            OP.add,
                        )
                    dum = dum_pool.tile([128, VS], F16, tag="dum")
                    nc.scalar.activation(
                        dum[:],
                        vp[m][:],
                        AF.Exp,
                        bias=negone[:, 0:1],
                        accum_out=sums[:, m : m + 1],
                    )

                def emit_group_sums(g):
                    ms = GROUP_MS[g]
                    lsum = stat.tile([128, len(ms)], F32, tag=f"lsum{g}")
                    nc.vector.tensor_scalar_sub(
                        lsum[:], sums[:, ms[0] : ms[-1] + 1], pad_sb[:, 0:1]
                    )
                    nc.sync.dma_start(out=sum_in[g][:], in_=lsum[:])
                    nc.gpsimd.collective_compute(
                        "AllReduce",
                        OP.add,
                        replica_groups=rg,
                        ins=[sum_in[g].opt()],
                        outs=[sum_out[g].opt()],
                    )

                def emit_group_finish(g):
                    ms = GROUP_MS[g]
                    gsum = stat.tile([128, len(ms)], F32, tag=f"gsum{g}")
                    nc.sync.dma_start(out=gsum[:], in_=sum_out[g][:])
                    logl = stat.tile([128, len(ms)], F32, tag=f"logl{g}")
                    nc.scalar.activation(logl[:], gsum[:], AF.Ln)
                    nc.vector.tensor_scalar(
                        nlogl1[:, ms[0] : ms[-1] + 1],
                        logl[:],
                        -1.0,
                        -1.0,
                        OP.mult,
                        OP.add,
                    )

                def emit_pass2_m(m):
                    # in-place normalize, then one whole-tile DMA (big rows)
                    nc.vector.tensor_scalar_add(
                        vp[m][:], vp[m][:], nlogl1[:, m : m + 1]
                    )
                    for rs in range(0, 128, 16):
                        nc.sync.dma_start(
                            out=out[m * 128 + rs : m * 128 + rs + 16, :],
                            in_=vp[m][rs : rs + 16, :],
                        )

                pend = []
                fin_q = [0]

                def maybe_finish(upto):
                    while fin_q[0] <= upto:
                        q = fin_q[0]
                        emit_group_finish(q)
                        pend.extend(GROUP_MS[q])
                        fin_q[0] += 1

                for g in range(NG):
                    for j, m in enumerate(GROUP_MS[g]):
                        emit_mtile(m)
                        if g == NG - 1 and j == 0:
                            maybe_finish(g - 1)
                        if j == len(GROUP_MS[g]) - 1:
                            emit_group_sums(g)
                            maybe_finish(g - 2)
                        if pend:
                            emit_pass2_m(pend.pop(0))
                maybe_finish(NG - 1)
                for m in pend:
                    emit_pass2_m(m)

    nc.compile()
    return nc


def bass_masks_identity(nc, ident_ap):
    from concourse import masks

    masks.make_identity(nc, ident_ap)


def _host_prep(inputs):
    """Per-core input maps from full inputs (numpy only)."""
    tok = np.asarray(inputs["token_ids"])
    typ = np.asarray(inputs["type_ids"])
    syn = np.asarray(inputs["synset_ids"])
    hw = np.asarray(inputs["highway"]).astype(bool)
    tok_emb = np.asarray(inputs["tok_emb"], dtype=np.float32)
    type_emb = np.asarray(inputs["type_emb"], dtype=np.float32)
    pos_emb = np.asarray(inputs["pos_emb"], dtype=np.float32)
    ln_g = np.asarray(inputs["ln_g"], dtype=np.float32)
    ln_b = np.asarray(inputs["ln_b"], dtype=np.float32)
    W = np.asarray(inputs["W"], dtype=np.float32)
    a = np.asarray(inputs["a"], dtype=np.float32)
    out_W = np.asarray(inputs["out_W"], dtype=np.float32)
    out_b = np.asarray(inputs["out_b"], dtype=np.float32)

    # embeddings (host gather + add, f32 like the reference)
    x_pre = tok_emb[tok] + type_emb[typ] + pos_emb[:N][None]  # (B,N,D)

    # graph mask (host index logic), transposed to [j, i], 1.0 = masked-out
    vis = syn[:, :, None] == syn[:, None, :]
    s1m = (typ == 1) & hw
    s3m = (typ == 3) & hw
    d1 = np.isin(typ, [0, 2, 5]) & hw
    d3 = np.isin(typ, [6, 4, 0]) & hw
    vis = vis | (s1m[:, :, None] & d1[:, None, :]) | (s3m[:, :, None] & d3[:, None, :])
    mask = vis & (tok != 0)[:, None, :]  # (B,N,N) over [i,j]
    maskt = (~mask).transpose(0, 2, 1).astype(np.uint8)  # (B,N,N) over [j,i]

    # GAT weights: fold ln_g, append a1/a2 contraction columns
    Wg = W * ln_g[None, :, None]  # (H,D,F)
    a1, a2 = a[:, :F], a[:, F:]
    c1 = np.einsum("hdf,hf->hd", Wg, a1)  # (H,D)
    c2 = np.einsum("hdf,hf->hd", Wg, a2)
    waug = np.concatenate([Wg, c1[:, :, None], c2[:, :, None]], axis=2)  # (H,D,130)
    waug = waug.transpose(1, 0, 2).reshape(D, H * 130).astype(np.float16)

    with_ln_b = bool(np.any(ln_b != 0.0))
    brow = None
    if with_ln_b:
        b1 = np.einsum("hdf,hf->hd", W, a1)
        b2 = np.einsum("hdf,hf->hd", W, a2)
        waug_b = np.concatenate([W, b1[:, :, None], b2[:, :, None]], axis=2)
        brow = np.einsum("d,hdc->hc", ln_b, waug_b).reshape(1, H * 130)
        brow = brow.astype(np.float16)

    # vocab shards of out_W^T (padded to 30528)
    wpad = np.zeros((VPAD, H * F), dtype=np.float32)
    wpad[:V] = out_W
    with_out_b = bool(np.any(out_b != 0.0))
    bpad = np.zeros((VPAD,), dtype=np.float32)
    bpad[:V] = out_b

    in_maps = []
    for c in range(NCORES):
        wc = wpad[c * VS : (c + 1) * VS].T.astype(np.float16)  # (512, VS)
        m = {
            "xpre": np.ascontiguousarray(x_pre[c]),
            "maskt": np.ascontiguousarray(maskt[c]),
            "waug": waug,
            "wst": np.ascontiguousarray(wc.reshape(4, 128, VS)),
            "pad": np.full(
                (128, 1),
                float(max(0, (c + 1) * VS - V)) if c == NCORES - 1 else 0.0,
                dtype=np.float32,
            ),
        }
        if with_ln_b:
            m["brow"] = brow
        if with_out_b:
            m["bvoc"] = np.ascontiguousarray(
                bpad[c * VS : (c + 1) * VS].reshape(1, VS).astype(np.float16)
            )
        in_maps.append(m)
    return in_maps, with_ln_b, with_out_b


def kernel(**inputs) -> np.ndarray:
    in_maps, with_ln_b, with_out_b = _host_prep(inputs)

    key = (with_ln_b, with_out_b)
    if key not in _NC_CACHE:
        _NC_CACHE[key] = _build(with_ln_b, with_out_b)
    nc = _NC_CACHE[key]

    trace = bool(int(os.environ.get("KBERT_TRACE", "0")))
    res = run_bass_kernel_spmd(
        nc, in_maps, core_ids=list(range(NCORES)), trace=trace
    )
    if trace and res.exec_time_ns is not None:
        print(f"HW exec time: {res.exec_time_ns} ns")
        if res.instructions_and_trace is not None:
            print(f"trace: {res.instructions_and_trace[1]}")

    full = np.empty((B * N, VPAD), dtype=np.float16)
    for c in range(NCORES):
        full[:, c * VS : (c + 1) * VS] = res.results[c]["out"]
    return np.ascontiguousarray(
        full[:, :V].reshape(B, N, V).astype(np.float32)
    )


# revision 23
# speedup vs baseline: 1.0073x; 1.0073x over previous
"""KBertGATEnricher Trainium2 kernel.

Sharding: data-parallel over batch (8 batches -> 8 cores) for embedding+GAT,
then AllGather of the f16 head features, then vocab-column-parallel output
Linear (f16 matmuls, kt-major so the PE streams back-to-back into all 8 PSUM
banks) + global log_softmax. The per-token sum-exp AllReduce is split into
groups so the normalize (in-place on vp) + f16 store tail overlaps compute;
the f16 output is upconverted to f32 on the host.

Self-contained: hardcodes all shapes; only imports the system-installed
concourse runtime.
"""

import os
import sys

sys.path.insert(0, "/opt/trn_rl_repo")

import numpy as np

from concourse import bass, bacc, mybir, tile
from concourse.bass_utils import run_bass_kernel_spmd

F32 = mybir.dt.float32
F16 = mybir.dt.float16
F8 = mybir.dt.float8e4
U8 = mybir.dt.uint8
PM_DR = mybir.MatmulPerfMode.DoubleRow

B, N, D, H, F, V = 8, 256, 768, 4, 128, 30522
NCORES = 8
VS = 3816          # per-core vocab columns (8*3816 = 30528, 6 pad cols)
VPAD = VS * NCORES
LN_EPS = 1e-12
ALPHA = 0.01       # leaky relu slope
MASK_NEG = -50.0   # masked attention logit (exact, LUT-safe)
NKT = D // 128     # 6 hidden k-tiles for the GAT matmuls
NM = (B * N) // 128  # 16 token m-tiles
CHUNKS = [(c0, min(512, VS - c0)) for c0 in range(0, VS, 512)]  # 8 chunks
WT = [min(1024, VS - t * 1024) for t in range(4)]  # zp tile widths
# AllReduce groups: big early (pipeline warmup), tiny last (short tail)
GROUP_MS = [[0, 1], [2, 3], [4, 5], [6, 7], [8, 9], [10, 11], [12, 13, 14, 15]]
NG = len(GROUP_MS)

AX = mybir.AxisListType
AF = mybir.ActivationFunctionType
OP = mybir.AluOpType

_NC_CACHE = {}


def _build(with_ln_b: bool, with_out_b: bool):
    """Build the SPMD Bass program (identical on all 8 cores)."""
    nc = bacc.Bacc(
        "TRN2",
        target_bir_lowering=False,
        debug=False,
        enable_asserts=False,
        num_devices=NCORES,
    )

    # ---- per-core I/O --------------------------------------------------
    xpre = nc.dram_tensor("xpre", [N, D], F32, kind="ExternalInput").ap()
    maskt = nc.dram_tensor("maskt", [N, N], U8, kind="ExternalInput").ap()
    waug = nc.dram_tensor("waug", [D, H * 130], F16, kind="ExternalInput").ap()
    wst = nc.dram_tensor("wst", [4, 128, VS], F16, kind="ExternalInput").ap()
    pad = nc.dram_tensor("pad", [128, 1], F32, kind="ExternalInput").ap()
    if with_ln_b:
        brow = nc.dram_tensor("brow", [1, H * 130], F16, kind="ExternalInput").ap()
    if with_out_b:
        bvoc = nc.dram_tensor("bvoc", [1, VS], F16, kind="ExternalInput").ap()
    out = nc.dram_tensor("out", [B * N, VS], F16, kind="ExternalOutput").ap()

    rg = [list(range(NCORES))]

    with tile.TileContext(nc) as tc:
        # ---- persistent SBUF ------------------------------------------
        with (
            tc.tile_pool(name="wpool", bufs=1) as wpool,
            tc.tile_pool(name="catf_pool", bufs=1) as catf_pool,
            tc.tile_pool(name="dram", bufs=1, space="DRAM") as dram,
        ):
            w_sb = [
                wpool.tile([128, VS], F16, tag=f"w{k}", name=f"w{k}")
                for k in range(4)
            ]
            for k in range(4):
                nc.sync.dma_start(out=w_sb[k][:], in_=wst[k, :, :])
            catf = [
                catf_pool.tile([128, B * N], F16, tag=f"catf{k}", name=f"catf{k}")
                for k in range(4)
            ]
            if with_out_b:
                bvoc_sb = wpool.tile([1, VS], F16, tag="bvoc")
                nc.sync.dma_start(out=bvoc_sb[:], in_=bvoc[:, :])
                ones1v = wpool.tile([1, 128], F16, tag="ones1v")
                nc.vector.memset(ones1v[:], 1.0)

            cc_in = dram.tile([H * F, N], F16, tag="cci", name="cci")
            cc_out = dram.tile([NCORES, H * F, N], F16, tag="cco", name="cco",
                               addr_space="Shared")
            sum_in = [
                dram.tile([128, len(GROUP_MS[g])], F32, tag=f"sin{g}", name=f"sin{g}")
                for g in range(NG)
            ]
            sum_out = [
                dram.tile([128, len(GROUP_MS[g])], F32, tag=f"sout{g}", name=f"sout{g}",
                          addr_space="Shared")
                for g in range(NG)
            ]

            # ==== phase A: embedding LN + GAT (own batch) ==============
            with (
                tc.tile_pool(name="pa", bufs=1) as pa,
                tc.tile_pool(name="pa_tmp", bufs=3) as pa_tmp,
                tc.tile_pool(name="ps_a", bufs=2, space="PSUM") as ps_a,
            ):
                idw = pa.tile([128, 128], F16, tag="idw")
                bass_masks_identity(nc, idw[:])
                ones1 = pa.tile([1, 128], F32, tag="ones1")
                nc.vector.memset(ones1[:], 1.0)
                if with_ln_b:
                    ones1h = pa.tile([1, 128], F16, tag="ones1h")
                    nc.vector.memset(ones1h[:], 1.0)
                    ones_n = pa.tile([1, N], F16, tag="ones_n")
                    nc.vector.memset(ones_n[:], 1.0)
                negt = pa.tile([128, H * N], F32, tag="negt")
                nc.vector.memset(negt[:], MASK_NEG)
                ones1f = pa.tile([1, 128], F16, tag="ones1f")
                nc.vector.memset(ones1f[:], 1.0)
                onesn2 = pa.tile([1, N], F16, tag="onesn2")
                nc.vector.memset(onesn2[:], 1.0)
                eps_sb = pa.tile([128, 1], F32, tag="eps_sb")
                nc.vector.memset(eps_sb[:], LN_EPS)

                waug_sb = [
                    pa.tile([128, H * 130], F16, tag=f"waug{kt}", name=f"waug{kt}") for kt in range(NKT)
                ]
                for kt in range(NKT):
                    nc.sync.dma_start(
                        out=waug_sb[kt][:], in_=waug[kt * 128 : (kt + 1) * 128, :]
                    )
                mask4 = [pa.tile([128, H * N], U8, tag=f"mask4_{j}", name=f"mask4_{j}") for j in range(2)]
                for jt in range(2):
                    for h in range(H):
                        nc.sync.dma_start(
                            out=mask4[jt][:, h * N : (h + 1) * N],
                            in_=maskt[jt * 128 : (jt + 1) * 128, :],
                        )
                if with_ln_b:
                    brow_sb = pa.tile([1, H * 130], F16, tag="brow")
                    nc.sync.dma_start(out=brow_sb[:], in_=brow[:, :])

                # ---- LayerNorm (tokens on partitions) -----------------
                xn_sb = [pa.tile([128, D], F16, tag=f"xn{m}", name=f"xn{m}") for m in range(2)]
                for m in range(2):
                    xp = pa_tmp.tile([128, D], F32, tag="xp")
                    nc.sync.dma_start(
                        out=xp[:], in_=xpre[m * 128 : (m + 1) * 128, :]
                    )
                    nmu = pa_tmp.tile([128, 1], F32, tag="nmu")
                    nc.vector.tensor_reduce(
                        out=nmu[:], in_=xp[:], axis=AX.X, op=OP.add, negate=True
                    )
                    nc.vector.tensor_scalar_mul(nmu[:], nmu[:], 1.0 / D)
                    xc = pa_tmp.tile([128, D], F32, tag="xc")
                    nc.vector.tensor_scalar_add(xc[:], xp[:], nmu[:, 0:1])
                    sq = pa_tmp.tile([128, D], F32, tag="sq")
                    ssum = pa_tmp.tile([128, 1], F32, tag="ssum")
                    nc.scalar.activation(
                        sq[:], xc[:], AF.Square, accum_out=ssum[:, 0:1]
                    )
                    sd = pa_tmp.tile([128, 1], F32, tag="sd")
                    nc.scalar.activation(
                        sd[:], ssum[:], AF.Sqrt, bias=eps_sb[:, 0:1], scale=1.0 / D
                    )
                    rstd = pa_tmp.tile([128, 1], F32, tag="rstd")
                    nc.vector.reciprocal(rstd[:], sd[:])
                    nc.vector.tensor_scalar_mul(xn_sb[m][:], xc[:], rstd[:, 0:1])

                # ---- transpose xn -> xT[kt] [128 hid, 256 tok] --------
                xt_sb = [pa.tile([128, N], F16, tag=f"xt{kt}", name=f"xt{kt}") for kt in range(NKT)]
                for kt in range(NKT):
                    for m in range(2):
                        ptr = ps_a.tile([128, 128], F16, tag="ptr")
                        nc.tensor.transpose(
                            ptr[:], xn_sb[m][:, kt * 128 : (kt + 1) * 128], idw[:]
                        )
                        nc.scalar.copy(
                            xt_sb[kt][:, m * 128 : (m + 1) * 128], ptr[:]
                        )

                # ---- per-head GAT -------------------------------------
                wh_sb = [
                    [pa.tile([128, 128], F16, tag=f"wh{h}_{m}", name=f"wh{h}_{m}") for m in range(2)]
                    for h in range(H)
                ]
                s1r = [pa.tile([1, N], F16, tag=f"s1r{h}", name=f"s1r{h}") for h in range(H)]
                s2r = [pa.tile([1, N], F16, tag=f"s2r{h}", name=f"s2r{h}") for h in range(H)]
                att = [
                    [pa.tile([128, N], F16, tag=f"att{h}_{m}", name=f"att{h}_{m}") for m in range(2)]
                    for h in range(H)
                ]
                cat_sb = [pa.tile([128, N], F16, tag=f"cat{h}", name=f"cat{h}") for h in range(H)]

                for h in range(H):
                    c0 = h * 130
                    # Wh (+ s1,s2 fused columns)
                    for m in range(2):
                        pwh = ps_a.tile([128, 130], F32, tag="pwh", bufs=1)
                        for kt in range(NKT):
                            nc.tensor.matmul(
                                pwh[:],
                                xt_sb[kt][:, m * 128 : (m + 1) * 128],
                                waug_sb[kt][:, c0 : c0 + 130],
                                start=(kt == 0),
                                stop=(kt == NKT - 1) and not with_ln_b,
                            )
                        if with_ln_b:
                            nc.tensor.matmul(
                                pwh[:],
                                ones1h[:],
                                brow_sb[:, c0 : c0 + 130],
                                start=False,
                                stop=True,
                            )
                        nc.scalar.copy(wh_sb[h][m][:], pwh[:, 0:128])

                    # s1 row: c1^T @ xT  -> [1, 256]
                    ps1 = ps_a.tile([1, N], F32, tag="ps1", bufs=1)
                    for kt in range(NKT):
                        nc.tensor.matmul(
                            ps1[:],
                            waug_sb[kt][:, c0 + 128 : c0 + 129],
                            xt_sb[kt][:],
                            start=(kt == 0),
                            stop=(kt == NKT - 1) and not with_ln_b,
                        )
                    if with_ln_b:
                        nc.tensor.matmul(
                            ps1[:],
                            brow_sb[:, c0 + 128 : c0 + 129],
                            ones_n[:],
                            start=False,
                            stop=True,
                        )
                    nc.scalar.copy(s1r[h][:], ps1[:])

                    # s2 row: c2^T @ xT -> [1, 256]
                    ps2 = ps_a.tile([1, N], F32, tag="ps2", bufs=1)
                    for kt in range(NKT):
                        nc.tensor.matmul(
                            ps2[:],
                            waug_sb[kt][:, c0 + 129 : c0 + 130],
                            xt_sb[kt][:],
                            start=(kt == 0),
                            stop=(kt == NKT - 1) and not with_ln_b,
                        )
                    if with_ln_b:
                        nc.tensor.matmul(
                            ps2[:],
                            brow_sb[:, c0 + 129 : c0 + 130],
                            ones_n[:],
                            start=False,
                            stop=True,
                        )
                    nc.scalar.copy(s2r[h][:], ps2[:])

                # ---- 4-head-wide attention softmax (no max-sub; e<=20) ----
                for jt in range(2):
                    petw = ps_a.tile([128, H * N], F32, tag="petw", bufs=1)
                    for h in range(H):
                        nc.tensor.matmul(
                            petw[:, h * N : (h + 1) * N],
                            ones1f[:],
                            s1r[h][:],
                            start=True,
                            stop=False,
                        )
                        nc.tensor.matmul(
                            petw[:, h * N : (h + 1) * N],
                            s2r[h][:, jt * 128 : (jt + 1) * 128],
                            onesn2[:],
                            start=False,
                            stop=True,
                        )
                    lrw = pa_tmp.tile([128, H * N], F32, tag="lrw")
                    nc.scalar.activation(lrw[:], petw[:], AF.Lrelu, alpha=ALPHA)
                    nc.vector.copy_predicated(lrw[:], mask4[jt][:], negt[:])
                    exw = pa_tmp.tile([128, H * N], F32, tag="exw")
                    nc.scalar.activation(exw[:], lrw[:], AF.Exp)
                    sums4 = pa_tmp.tile([128, H], F32, tag="sums4")
                    nc.vector.tensor_reduce(
                        out=sums4[:],
                        in_=exw[:].rearrange("p (b i) -> p b i", b=H),
                        axis=AX.X,
                        op=OP.add,
                    )
                    rec4 = pa_tmp.tile([128, H], F32, tag="rec4")
                    nc.vector.reciprocal(rec4[:], sums4[:])
                    for h in range(H):
                        nc.vector.tensor_scalar_mul(
                            att[h][jt][:], exw[:, h * N : (h + 1) * N], rec4[:, h : h + 1]
                        )

                for h in range(H):
                    # hp^T = Wh^T @ att^T, then elu -> catT rows of head h
                    php = ps_a.tile([128, N], F32, tag="php", bufs=1)
                    for jt in range(2):
                        nc.tensor.matmul(
                            php[:],
                            wh_sb[h][jt][:],
                            att[h][jt][:],
                            start=(jt == 0),
                            stop=(jt == 1),
                        )
                    hneg = pa_tmp.tile([128, N], F32, tag="hneg")
                    nc.vector.tensor_scalar_min(hneg[:], php[:], 0.0)
                    he = pa_tmp.tile([128, N], F32, tag="he")
                    nc.scalar.activation(he[:], hneg[:], AF.Exp)
                    r1 = pa_tmp.tile([128, N], F32, tag="r1")
                    nc.vector.tensor_scalar(r1[:], he[:], -1.0, 1.0, OP.mult, OP.add)
                    nc.vector.scalar_tensor_tensor(
                        cat_sb[h][:], php[:], 0.0, r1[:], OP.max, OP.subtract
                    )
                    nc.sync.dma_start(
                        out=cc_in[h * 128 : (h + 1) * 128, :], in_=cat_sb[h][:]
                    )

            # ==== AllGather cat, assemble GEMM lhsT tiles ==============
            nc.gpsimd.collective_compute(
                "AllGather",
                OP.bypass,
                replica_groups=rg,
                ins=[cc_in.opt()],
                outs=[cc_out.opt()],
            )
            for kt in range(4):
                for r in range(NCORES):
                    nc.sync.dma_start(
                        out=catf[kt][:, r * N : (r + 1) * N],
                        in_=cc_out[r, kt * 128 : (kt + 1) * 128, :],
                    )

            # ==== vocab-parallel output linear + softmax stats =========
            with (
                tc.tile_pool(name="vp_pool", bufs=14) as vp_pool,
                tc.tile_pool(name="wide_tmp", bufs=2) as wide_tmp,
                tc.tile_pool(name="dum_pool", bufs=2) as dum_pool,
                tc.tile_pool(name="stat", bufs=1) as stat,
                tc.tile_pool(name="ps_z", bufs=4, space="PSUM") as ps_z,
            ):
                vp = {}
                sums = stat.tile([128, NM], F32, tag="sums")
                negone = stat.tile([128, 1], F32, tag="negone")
                nc.vector.memset(negone[:], -1.0)
                pad_sb = stat.tile([128, 1], F32, tag="pad_sb")
                nc.sync.dma_start(out=pad_sb[:], in_=pad[:, :])
                nlogl1 = stat.tile([128, NM], F32, tag="nlogl1")

                def emit_mtile(m):
                    vp[m] = vp_pool.tile([128, VS], F16, tag="vp", name=f"vp{m}")
                    zps = [ps_z.tile([128, 1024], F32, tag="zp", name=f"zp{m}_{t}") for t in range(4)]
                    for kt in range(4):
                        for ci, (c0, cw) in enumerate(CHUNKS):
                            t, h = divmod(ci, 2)
                            nc.tensor.matmul(
                                zps[t][:, h * 512 : h * 512 + cw],
                                catf[kt][:, m * 128 : (m + 1) * 128],
                                w_sb[kt][:, c0 : c0 + cw],
                                start=(kt == 0),
                                stop=(kt == 3) and not with_out_b,
                            )
                    if with_out_b:
                        for ci, (c0, cw) in enumerate(CHUNKS):
                            t, h = divmod(ci, 2)
                            nc.tensor.matmul(
                                zps[t][:, h * 512 : h * 512 + cw],
                                ones1v[:],
                                bvoc_sb[:, c0 : c0 + cw],
                                start=False,
                                stop=True,
                            )
                    for t in range(4):
                        w = WT[t]
                        e0 = wide_tmp.tile([128, 1024], F16, tag="e0")
                        nc.scalar.activation(e0[:, 0:w], zps[t][:, 0:w], AF.Exp)
                        tmin = wide_tmp.tile([128, 1024], F16, tag="tmin")
                        nc.vector.tensor_scalar_min(tmin[:, 0:w], e0[:, 0:w], 1.0)
                        nc.vector.scalar_tensor_tensor(
                            vp[m][:, t * 1024 : t * 1024 + w],
                            zps[t][:, 0:w],
                            0.0,
                            tmin[:, 0:w],
                            OP.max,
                            OP.add,
                        )
                    dum = dum_pool.tile([128, VS], F16, tag="dum")
                    nc.scalar.activation(
                        dum[:],
                        vp[m][:],
                        AF.Exp,
                        bias=negone[:, 0:1],
                        accum_out=sums[:, m : m + 1],
                    )

                def emit_group_sums(g):
                    ms = GROUP_MS[g]
                    lsum = stat.tile([128, len(ms)], F32, tag=f"lsum{g}")
                    nc.vector.tensor_scalar_sub(
                        lsum[:], sums[:, ms[0] : ms[-1] + 1], pad_sb[:, 0:1]
                    )
                    nc.sync.dma_start(out=sum_in[g][:], in_=lsum[:])
                    nc.gpsimd.collective_compute(
                        "AllReduce",
                        OP.add,
                        replica_groups=rg,
                        ins=[sum_in[g].opt()],
                        outs=[sum_out[g].opt()],
                    )

                def emit_group_finish(g):
                    ms = GROUP_MS[g]
                    gsum = stat.tile([128, len(ms)], F32, tag=f"gsum{g}")
                    nc.sync.dma_start(out=gsum[:], in_=sum_out[g][:])
                    logl = stat.tile([128, len(ms)], F32, tag=f"logl{g}")
                    nc.scalar.activation(logl[:], gsum[:], AF.Ln)
                    nc.vector.tensor_scalar(
                        nlogl1[:, ms[0] : ms[-1] + 1],
                        logl[:],
                        -1.0,
                        -1.0,
                        OP.mult,
                        OP.add,
                    )

                def emit_pass2_m(m):
                    # in-place normalize, then one whole-tile DMA (big rows)
                    nc.vector.tensor_scalar_add(
                        vp[m][:], vp[m][:], nlogl1[:, m : m + 1]
                    )
                    for rs in range(0, 128, 16):
                        nc.sync.dma_start(
                            out=out[m * 128 + rs : m * 128 + rs + 16, :],
                            in_=vp[m][rs : rs + 16, :],
                        )

                pend = []
                fin_q = [0]

                def maybe_finish(upto):
                    while fin_q[0] <= upto:
                        q = fin_q[0]
                        emit_group_finish(q)
                        pend.extend(GROUP_MS[q])
                        fin_q[0] += 1

                for g in range(NG):
                    for j, m in enumerate(GROUP_MS[g]):
                        emit_mtile(m)
                        if g == NG - 1 and j == 0:
                            maybe_finish(g - 1)
                        if j == len(GROUP_MS[g]) - 1:
                            emit_group_sums(g)
                            maybe_finish(g - 2)
                        if pend:
                            emit_pass2_m(pend.pop(0))
                maybe_finish(NG - 1)
                for m in pend:
                    emit_pass2_m(m)

    nc.compile()
    return nc


def bass_masks_identity(nc, ident_ap):
    from concourse import masks

    masks.make_identity(nc, ident_ap)


def _host_prep(inputs):
    """Per-core input maps from full inputs (numpy only)."""
    tok = np.asarray(inputs["token_ids"])
    typ = np.asarray(inputs["type_ids"])
    syn = np.asarray(inputs["synset_ids"])
    hw = np.asarray(inputs["highway"]).astype(bool)
    tok_emb = np.asarray(inputs["tok_emb"], dtype=np.float32)
    type_emb = np.asarray(inputs["type_emb"], dtype=np.float32)
    pos_emb = np.asarray(inputs["pos_emb"], dtype=np.float32)
    ln_g = np.asarray(inputs["ln_g"], dtype=np.float32)
    ln_b = np.asarray(inputs["ln_b"], dtype=np.float32)
    W = np.asarray(inputs["W"], dtype=np.float32)
    a = np.asarray(inputs["a"], dtype=np.float32)
    out_W = np.asarray(inputs["out_W"], dtype=np.float32)
    out_b = np.asarray(inputs["out_b"], dtype=np.float32)

    # embeddings (host gather + add, f32 like the reference)
    x_pre = tok_emb[tok] + type_emb[typ] + pos_emb[:N][None]  # (B,N,D)

    # graph mask (host index logic), transposed to [j, i], 1.0 = masked-out
    vis = syn[:, :, None] == syn[:, None, :]
    s1m = (typ == 1) & hw
    s3m = (typ == 3) & hw
    d1 = np.isin(typ, [0, 2, 5]) & hw
    d3 = np.isin(typ, [6, 4, 0]) & hw
    vis = vis | (s1m[:, :, None] & d1[:, None, :]) | (s3m[:, :, None] & d3[:, None, :])
    mask = vis & (tok != 0)[:, None, :]  # (B,N,N) over [i,j]
    maskt = (~mask).transpose(0, 2, 1).astype(np.uint8)  # (B,N,N) over [j,i]

    # GAT weights: fold ln_g, append a1/a2 contraction columns
    Wg = W * ln_g[None, :, None]  # (H,D,F)
    a1, a2 = a[:, :F], a[:, F:]
    c1 = np.einsum("hdf,hf->hd", Wg, a1)  # (H,D)
    c2 = np.einsum("hdf,hf->hd", Wg, a2)
    waug = np.concatenate([Wg, c1[:, :, None], c2[:, :, None]], axis=2)  # (H,D,130)
    waug = waug.transpose(1, 0, 2).reshape(D, H * 130).astype(np.float16)

    with_ln_b = bool(np.any(ln_b != 0.0))
    brow = None
    if with_ln_b:
        b1 = np.einsum("hdf,hf->hd", W, a1)
        b2 = np.einsum("hdf,hf->hd", W, a2)
        waug_b = np.concatenate([W, b1[:, :, None], b2[:, :, None]], axis=2)
        brow = np.einsum("d,hdc->hc", ln_b, waug_b).reshape(1, H * 130)
        brow = brow.astype(np.float16)

    # vocab shards of out_W^T (padded to 30528)
    wpad = np.zeros((VPAD, H * F), dtype=np.float32)
    wpad[:V] = out_W
    with_out_b = bool(np.any(out_b != 0.0))
    bpad = np.zeros((VPAD,), dtype=np.float32)
    bpad[:V] = out_b

    in_maps = []
    for c in range(NCORES):
        wc = wpad[c * VS : (c + 1) * VS].T.astype(np.float16)  # (512, VS)
        m = {
            "xpre": np.ascontiguousarray(x_pre[c]),
            "maskt": np.ascontiguousarray(maskt[c]),
            "waug": waug,
            "wst": np.ascontiguousarray(wc.reshape(4, 128, VS)),
            "pad": np.full(
                (128, 1),
                float(max(0, (c + 1) * VS - V)) if c == NCORES - 1 else 0.0,
                dtype=np.float32,
            ),
        }
        if with_ln_b:
            m["brow"] = brow
        if with_out_b:
            m["bvoc"] = np.ascontiguousarray(
                bpad[c * VS : (c + 1) * VS].reshape(1, VS).astype(np.float16)
            )
        in_maps.append(m)
    return in_maps, with_ln_b, with_out_b


def kernel(**inputs) -> np.ndarray:
    in_maps, with_ln_b, with_out_b = _host_prep(inputs)

    key = (with_ln_b, with_out_b)
    if key not in _NC_CACHE:
        _NC_CACHE[key] = _build(with_ln_b, with_out_b)
    nc = _NC_CACHE[key]

    trace = bool(int(os.environ.get("KBERT_TRACE", "0")))
    res = run_bass_kernel_spmd(
        nc, in_maps, core_ids=list(range(NCORES)), trace=trace
    )
    if trace and res.exec_time_ns is not None:
        print(f"HW exec time: {res.exec_time_ns} ns")
        if res.instructions_and_trace is not None:
            print(f"trace: {res.instructions_and_trace[1]}")

    full = np.empty((B * N, VPAD), dtype=np.float16)
    for c in range(NCORES):
        full[:, c * VS : (c + 1) * VS] = res.results[c]["out"]
    return np.ascontiguousarray(
        full[:, :V].reshape(B, N, V).astype(np.float32)
    )
